# revision 6
# baseline (speedup 1.0000x reference)
"""ComplEx decoder scoring kernel for 8 Trainium2 NeuronCores.

score[e] = sum_h Re( (s_e * r_{t_e}) * conj(d_e) )  over L2-normalized node
rows, computed as raw_dot(s,d,r) / sqrt(|s|^2 * |d|^2).

Strategy: shard the 300k edges across 8 cores data-parallel; replicate z and
the relation table. Node rows are fetched with the InstDMAGatherAnt SWDGE
gather (int16 indices). To fit int16, nodes are split into 4 blocks of 25000
rows and each core's edges are bucketed by (src_block, dst_block); indices
are block-local. Every bucket is padded (with index 0) to a cross-core
common capacity so one SPMD program serves all cores; the host un-permutes
the per-bucket scores back to edge order.

Per 1024-edge chunk, three dma_gathers (src rows, dst rows, rel rows) land
edge k at partition k%128, slot k//128. DVE computes the complex products,
ACT computes row norms (Square+accum) and the dot reduction (Copy+accum).
"""

import os
import sys

for _p in ("/root/.axon_site", "/root/.axon_site/_ro/trn_rl_repo",
           "/root/.axon_site/_ro/pypackages", "/opt/trn_rl_repo"):
    if os.path.isdir(_p) and _p not in sys.path:
        sys.path.append(_p)

import numpy as np

import concourse.bacc as bacc
import concourse.bass as bass
import concourse.mybir as mybir
from concourse.bass_utils import run_bass_kernel_spmd
from concourse.tile import TileContext

F32 = mybir.dt.float32
I16 = mybir.dt.int16
AX = mybir.AxisListType
ALU = mybir.AluOpType
ACTF = mybir.ActivationFunctionType

# Problem constants (hardcoded per contract).
N_NODES = 100000
HID = 512
HH = HID // 2
N_REL = 500
N_EDGES = 300000
N_CORES = 8

P = 128
NBLK = 4                   # node blocks (block size 25000 fits int16)
BS = N_NODES // NBLK
NBUCK = NBLK * NBLK
EPC = N_EDGES // N_CORES   # 37500 edges per core
CHUNK = 1024               # max edges per dma_gather call


def _wrap_idx(idx):
    """[n] int16 -> [128, n//16] wrapped (i at [i%16, i//16]), replicated x8."""
    n = idx.shape[0]
    w = idx.reshape(n // 16, 16).T  # [16, n//16]
    return np.tile(w, (8, 1)).astype(np.int16)


def plan_and_pack(edge_index, edge_type):
    """Bucket/sort each core's edges; compute shared capacities; pack idx
    arrays. Returns (caps, per-core in_map idx arrays, recover info)."""
    src_all = np.asarray(edge_index[0]).astype(np.int64)
    dst_all = np.asarray(edge_index[1]).astype(np.int64)
    rel_all = np.asarray(edge_type).astype(np.int64)

    orders, counts = [], []
    for c in range(N_CORES):
        lo, hi = c * EPC, (c + 1) * EPC
        b = (src_all[lo:hi] // BS) * NBLK + dst_all[lo:hi] // BS
        order = np.argsort(b, kind="stable")
        orders.append(order)
        counts.append(np.bincount(b, minlength=NBUCK))
    counts = np.stack(counts)  # [cores, NBUCK]
    caps = (np.maximum(counts.max(axis=0), 1) + 127) // 128 * 128  # [NBUCK]

    packs, recovers = [], []
    for c in range(N_CORES):
        lo = c * EPC
        order = orders[c]
        src = src_all[lo + order]
        dst = dst_all[lo + order]
        rel = rel_all[lo + order]
        cnt = counts[c]
        starts = np.concatenate([[0], np.cumsum(cnt)])
        si, di, ri = [], [], []
        # recover: for each original edge position, its (partition, slot)
        slot_off = np.concatenate([[0], np.cumsum(caps // 128)])
        part_of = np.empty(EPC, np.int64)
        slot_of = np.empty(EPC, np.int64)
        for b in range(NBUCK):
            k0, k1 = starts[b], starts[b + 1]
            n, cap = k1 - k0, caps[b]
            s_loc = np.zeros(cap, np.int16)
            d_loc = np.zeros(cap, np.int16)
            r_loc = np.zeros(cap, np.int16)
            s_loc[:n] = (src[k0:k1] % BS).astype(np.int16)
            d_loc[:n] = (dst[k0:k1] % BS).astype(np.int16)
            r_loc[:n] = rel[k0:k1].astype(np.int16)
            si.append(s_loc)
            di.append(d_loc)
            ri.append(r_loc)
            kk = np.arange(n)
            part_of[k0:k1] = kk % 128
            slot_of[k0:k1] = slot_off[b] + kk // 128
        # chunk-wise wrapping, concatenated along columns
        def pack(parts):
            flat = np.concatenate(parts)
            cols = []
            pos = 0
            for b in range(NBUCK):
                cap = caps[b]
                for c0 in range(0, cap, CHUNK):
                    n = min(CHUNK, cap - c0)
                    cols.append(_wrap_idx(flat[pos:pos + n]))
                    pos += n
            return np.ascontiguousarray(np.concatenate(cols, axis=1))
        packs.append({
            "idx_src": pack(si), "idx_dst": pack(di), "idx_rel": pack(ri),
        })
        # inverse permutation: original edge i -> (part, slot)
        inv_part = np.empty(EPC, np.int64)
        inv_slot = np.empty(EPC, np.int64)
        inv_part[order] = part_of
        inv_slot[order] = slot_of
        recovers.append((inv_part, inv_slot))
    return caps, packs, recovers


def build_nc(caps):
    nc = bacc.Bacc()
    slot_off = np.concatenate([[0], np.cumsum(caps // 128)])
    S = int(slot_off[-1])
    COLS = int(caps.sum() // 16)

    z_d = nc.dram_tensor("z", [N_NODES, HID], F32, kind="ExternalInput")
    rel_d = nc.dram_tensor("relcat", [N_REL, HID], F32, kind="ExternalInput")
    isrc_d = nc.dram_tensor("idx_src", [P, COLS], I16, kind="ExternalInput")
    idst_d = nc.dram_tensor("idx_dst", [P, COLS], I16, kind="ExternalInput")
    irel_d = nc.dram_tensor("idx_rel", [P, COLS], I16, kind="ExternalInput")
    out_d = nc.dram_tensor("scores", [P, S], F32, kind="ExternalOutput")

    with TileContext(nc) as tc:
        with (
            tc.tile_pool(name="persist", bufs=1) as persist,
            tc.tile_pool(name="gath", bufs=2) as gath,
            tc.tile_pool(name="scratch", bufs=2) as scratch,
            tc.tile_pool(name="actscr", bufs=2) as actscr,
            tc.tile_pool(name="small", bufs=3) as small,
        ):
            isrc_t = persist.tile([P, COLS], I16)
            nc.sync.dma_start(out=isrc_t[:], in_=isrc_d[:])
            idst_t = persist.tile([P, COLS], I16)
            nc.sync.dma_start(out=idst_t[:], in_=idst_d[:])
            irel_t = persist.tile([P, COLS], I16)
            nc.sync.dma_start(out=irel_t[:], in_=irel_d[:])
            scores_t = persist.tile([P, S], F32)

            col = 0
            for b in range(NBUCK):
                blk_s, blk_d = b // NBLK, b % NBLK
                z_s = z_d[blk_s * BS:(blk_s + 1) * BS, :]
                z_dd = z_d[blk_d * BS:(blk_d + 1) * BS, :]
                cap = int(caps[b])
                g_off = int(slot_off[b])
                for c0 in range(0, cap, CHUNK):
                    n = min(CHUNK, cap - c0)
                    slots = n // 128
                    cols = n // 16
                    st = gath.tile([P, slots, HID], F32, tag="st")
                    nc.gpsimd.dma_gather(
                        st[:], z_s, isrc_t[:, col:col + cols], n, n, HID)
                    dt_ = gath.tile([P, slots, HID], F32, tag="dt")
                    nc.gpsimd.dma_gather(
                        dt_[:], z_dd, idst_t[:, col:col + cols], n, n, HID)
                    rt = gath.tile([P, slots, HID], F32, tag="rt")
                    nc.gpsimd.dma_gather(
                        rt[:], rel_d[:], irel_t[:, col:col + cols], n, n, HID)

                    ns = small.tile([P, slots], F32, tag="ns")
                    nd = small.tile([P, slots], F32, tag="nd")
                    raw = small.tile([P, slots], F32, tag="raw")

                    G4 = 4
                    for h0 in range(0, slots, G4):
                        g = min(G4, slots - h0)
                        sl = slice(h0, h0 + g)
                        s4, d4, r4 = st[:, sl, :], dt_[:, sl, :], rt[:, sl, :]

                        sd4 = scratch.tile([P, G4, HID], F32, tag="sd4")
                        nc.vector.tensor_mul(sd4[:, :g], s4, d4)
                        pq4 = scratch.tile([P, G4, HID], F32, tag="pq4")
                        nc.vector.tensor_add(
                            pq4[:, :g, 0:HH], sd4[:, :g, 0:HH],
                            sd4[:, :g, HH:HID])
                        c1 = scratch.tile([P, G4, HH], F32, tag="c1")
                        nc.vector.tensor_mul(
                            c1[:, :g], s4[:, :, 0:HH], d4[:, :, HH:HID])
                        c2 = scratch.tile([P, G4, HH], F32, tag="c2")
                        nc.vector.tensor_mul(
                            c2[:, :g], s4[:, :, HH:HID], d4[:, :, 0:HH])
                        nc.vector.tensor_sub(
                            pq4[:, :g, HH:HID], c1[:, :g], c2[:, :g])
                        prod4 = scratch.tile([P, G4, HID], F32, tag="prod4")
                        nc.vector.tensor_mul(prod4[:, :g], pq4[:, :g], r4)

                        for j in range(g):
                            jj = h0 + j
                            a1 = actscr.tile([P, HID], F32, tag="a1")
                            nc.scalar.activation(
                                a1[:], st[:, jj, :], ACTF.Square,
                                accum_out=ns[:, jj:jj + 1])
                            a2 = actscr.tile([P, HID], F32, tag="a2")
                            nc.scalar.activation(
                                a2[:], dt_[:, jj, :], ACTF.Square,
                                accum_out=nd[:, jj:jj + 1])
                            a3 = actscr.tile([P, HID], F32, tag="a3")
                            nc.scalar.activation(
                                a3[:], prod4[:, j, :], ACTF.Copy,
                                accum_out=raw[:, jj:jj + 1])

                    den = small.tile([P, slots], F32, tag="den")
                    nc.vector.tensor_mul(den[:], ns[:], nd[:])
                    denb = small.tile([P, slots], F32, tag="denb")
                    nc.vector.tensor_scalar_max(denb[:], den[:], 1e-24)
                    sq = small.tile([P, slots], F32, tag="sq")
                    nc.scalar.activation(sq[:], denb[:], ACTF.Sqrt)
                    rc = small.tile([P, slots], F32, tag="rc")
                    nc.vector.reciprocal(rc[:], sq[:])
                    nc.vector.tensor_mul(
                        scores_t[:, g_off + c0 // 128:g_off + c0 // 128 + slots],
                        raw[:], rc[:])
                    col += cols

            nc.sync.dma_start(out=out_d[:], in_=scores_t[:])

    nc.finalize()
    return nc


_NC_CACHE = {}


def get_nc(caps):
    key = tuple(int(x) for x in caps)
    if key not in _NC_CACHE:
        _NC_CACHE.clear()
        _NC_CACHE[key] = build_nc(caps)
    return _NC_CACHE[key]


def kernel(z, edge_index, edge_type, rel_re, rel_im):
    z = np.ascontiguousarray(np.asarray(z, np.float32))
    relcat = np.ascontiguousarray(
        np.concatenate(
            [np.asarray(rel_re, np.float32), np.asarray(rel_im, np.float32)],
            axis=1))

    caps, packs, recovers = plan_and_pack(edge_index, edge_type)
    nc = get_nc(caps)
    in_maps = [
        {"z": z, "relcat": relcat, **packs[c]} for c in range(N_CORES)
    ]
    res = run_bass_kernel_spmd(nc, in_maps, core_ids=list(range(N_CORES)))
    outs = []
    for c in range(N_CORES):
        sc = np.asarray(res.results[c]["scores"], np.float32)
        inv_part, inv_slot = recovers[c]
        outs.append(sc[inv_part, inv_slot])
    return np.concatenate(outs)
